# revision 6
# baseline (speedup 1.0000x reference)
"""Bass/Tile TRN2 kernel for nn_Link_83047487635827 (gnn_message_passing).

Math (verified against the reference):
  reference(inputs, tag_to_token, gat_mask) decomposes as
    binary = (tag_to_token > 0)                       # (T, N)
    temp   = relu(C^T @ binary),  C = I - strict_lower_ones(T)
           # == relu(binary_i - sum_{j>i} binary_j), the "direct string" map
    r      = rowsum(temp)                             # (T,)
    P      = temp @ inputs                            # (T, D)
    child  == gat_mask  (the reference deduce_child loop is an identity
              for 0/1 masks: rows j>i are still zero when read)
    out    = (I - S_up)^{-1} @ L_low @ diag(1/r) @ P
           # S_up/L_low = strict-upper / inclusive-lower parts of child;
           # the sequential T-step recurrence is exactly this triangular solve
    (I - S_up)^{-1} = prod_{k=0..6} (I + S_up^(2^k))   # S_up nilpotent

Sharding: the contraction (token) dimension N is split across the 8 cores
(16 MiB of `inputs` + 2 MiB of tag_to_token per core instead of replicating
the 16 MiB tag_to_token 8x as D-sharding would). Each core computes a
partial (P, r); one small (513 KiB) AllReduce combines them; every core
then (redundantly, ~2 us) applies the recurrence matrix and writes the
full (T, D) output. Host returns core 0's output.
"""

import numpy as np

B, T, N, D = 1, 128, 32768, 1024
NCORES = 8
NS = N // NCORES          # tokens per core = 4096
CHUNK = 512               # t2t tokens per pipeline chunk
NCHUNK = NS // CHUNK      # 8
SUBS = CHUNK // 128       # 4 token-subtiles of 128 per chunk
NSUB = NS // 128          # 32 subtiles total per core
CC_LEN = T * D + T        # flat collective buffer: P then r

_PROGRAM = {}             # with_cc -> nc cache — compile once per process


def _host_consts():
    f32 = np.float32
    ident = np.eye(T, dtype=f32)
    # C[j, i] = 1 if j == i, -1 if j > i  (temp^T tile = binary_tile^T @ C)
    cmat = np.eye(T, dtype=f32) - np.tril(np.ones((T, T), dtype=f32), -1)
    msl = np.tril(np.ones((T, T), dtype=f32), -1)   # strict lower
    msu = np.triu(np.ones((T, T), dtype=f32), 1)    # strict upper
    mle = np.tril(np.ones((T, T), dtype=f32), 0)    # lower inclusive
    ones_col = np.ones((T, 1), dtype=f32)
    return {
        "ident": ident, "cmat": cmat, "msl": msl,
        "msu": msu, "mle": mle, "onescol": ones_col,
    }


def _build_program(with_cc=True):
    import concourse.bacc as bacc
    import concourse.bass as bass
    import concourse.mybir as mybir
    import concourse.tile as tile
    from concourse.bass import ts

    f32 = mybir.dt.float32
    i32 = mybir.dt.int32
    Alu = mybir.AluOpType

    nc = bacc.Bacc(
        "TRN2", target_bir_lowering=False, debug=False, num_devices=NCORES
    )

    x_d = nc.dram_tensor("x", (NS, D), f32, kind="ExternalInput")
    t2t_d = nc.dram_tensor("t2t", (T, NS), f32, kind="ExternalInput")
    gm_d = nc.dram_tensor("gm", (T, T), i32, kind="ExternalInput")
    ident_d = nc.dram_tensor("ident", (T, T), f32, kind="ExternalInput")
    cmat_d = nc.dram_tensor("cmat", (T, T), f32, kind="ExternalInput")
    msl_d = nc.dram_tensor("msl", (T, T), f32, kind="ExternalInput")
    msu_d = nc.dram_tensor("msu", (T, T), f32, kind="ExternalInput")
    mle_d = nc.dram_tensor("mle", (T, T), f32, kind="ExternalInput")
    ones_d = nc.dram_tensor("onescol", (T, 1), f32, kind="ExternalInput")
    out_d = nc.dram_tensor("out", (T, D), f32, kind="ExternalOutput")

    with tile.TileContext(nc) as tc:
        with (
            tc.tile_pool(name="const", bufs=1) as constp,
            tc.tile_pool(name="xin", bufs=8) as xp,
            tc.tile_pool(name="t2tin", bufs=3) as t2tp,
            tc.tile_pool(name="work", bufs=3) as workp,
            tc.tile_pool(name="mchain", bufs=2) as mp,
            tc.tile_pool(name="psacc", bufs=1, space=bass.MemorySpace.PSUM) as psA,
            tc.tile_pool(name="pstt", bufs=2, space=bass.MemorySpace.PSUM) as psB,
            tc.tile_pool(name="psm", bufs=3, space=bass.MemorySpace.PSUM) as psM,
            tc.tile_pool(name="dram", bufs=1, space=bass.MemorySpace.DRAM) as dramp,
        ):
            # ---- constants ----
            ident = constp.tile([T, T], f32, tag="ident")
            nc.sync.dma_start(ident[:], ident_d[:])
            cmat = constp.tile([T, T], f32, tag="cmat")
            nc.sync.dma_start(cmat[:], cmat_d[:])
            msl = constp.tile([T, T], f32, tag="msl")
            nc.sync.dma_start(msl[:], msl_d[:])
            msu = constp.tile([T, T], f32, tag="msu")
            nc.sync.dma_start(msu[:], msu_d[:])
            mle = constp.tile([T, T], f32, tag="mle")
            nc.sync.dma_start(mle[:], mle_d[:])
            onescol = constp.tile([T, 1], f32, tag="onescol")
            nc.sync.dma_start(onescol[:], ones_d[:])
            gm_i = constp.tile([T, T], i32, tag="gmi")
            nc.sync.dma_start(gm_i[:], gm_d[:])
            gm_f = constp.tile([T, T], f32, tag="gmf")
            nc.vector.tensor_copy(gm_f[:], gm_i[:])

            # ---- recurrence matrix chain (tiny; overlaps the stream loop) ----
            # gmT = gm^T via PE transpose
            gmT_ps = psM.tile([T, T], f32, tag="mm")
            nc.tensor.transpose(gmT_ps[:], gm_f[:], ident[:])
            gmT = constp.tile([T, T], f32, tag="gmT")
            nc.vector.tensor_copy(gmT[:], gmT_ps[:])

            # Tp = S_up^T = gmT * strict_lower ; TpT = S_up = gm * strict_upper
            Tp = mp.tile([T, T], f32, tag="Tp")
            nc.vector.tensor_tensor(out=Tp[:], in0=gmT[:], in1=msl[:], op=Alu.mult)
            TpT = mp.tile([T, T], f32, tag="TpT")
            nc.vector.tensor_tensor(out=TpT[:], in0=gm_f[:], in1=msu[:], op=Alu.mult)
            # G = I + Tp ; after the chain G == ((I - S_up)^{-1})^T
            G = mp.tile([T, T], f32, tag="G")
            nc.vector.tensor_tensor(out=G[:], in0=ident[:], in1=Tp[:], op=Alu.add)
            L_low = constp.tile([T, T], f32, tag="Llow")
            nc.vector.tensor_tensor(out=L_low[:], in0=gm_f[:], in1=mle[:], op=Alu.mult)

            for _k in range(1, 7):
                # matmul(out, lhsT, rhs) = lhsT.T @ rhs
                sq_ps = psM.tile([T, T], f32, tag="mm")
                nc.tensor.matmul(sq_ps[:], Tp[:], TpT[:])      # TpT @ TpT = (Tp^2)^T
                sq2_ps = psM.tile([T, T], f32, tag="mm")
                nc.tensor.matmul(sq2_ps[:], TpT[:], Tp[:])     # Tp @ Tp
                Tp_n = mp.tile([T, T], f32, tag="Tp")
                nc.vector.tensor_copy(Tp_n[:], sq2_ps[:])
                TpT_n = mp.tile([T, T], f32, tag="TpT")
                nc.vector.tensor_copy(TpT_n[:], sq_ps[:])
                gu_ps = psM.tile([T, T], f32, tag="mm")
                nc.tensor.matmul(gu_ps[:], TpT_n[:], G[:])     # Tp^2 @ G
                G_n = mp.tile([T, T], f32, tag="G")
                nc.vector.tensor_tensor(out=G_n[:], in0=G[:], in1=gu_ps[:], op=Alu.add)
                Tp, TpT, G = Tp_n, TpT_n, G_n

            # MT = M^T = L_low^T @ G  (lhsT = L_low)
            mt_ps = psM.tile([T, T], f32, tag="mm")
            nc.tensor.matmul(mt_ps[:], L_low[:], G[:])
            MT = constp.tile([T, T], f32, tag="MT")
            nc.vector.tensor_copy(MT[:], mt_ps[:])

            # ---- streaming phase: P += temp^T.T @ x, r += colsum(temp^T) ----
            p0 = psA.tile([128, 512], f32, tag="p0")
            p1 = psA.tile([128, 512], f32, tag="p1")
            r_ps = psA.tile([1, 128], f32, tag="r")

            for j in range(NCHUNK):
                tt_in = t2tp.tile([T, CHUNK], f32, tag="ttin")
                nc.sync.dma_start(tt_in[:], t2t_d[:, ts(j, CHUNK)])
                # binarize in place: (t2t > 0) -> 1.0/0.0
                nc.vector.tensor_scalar(
                    tt_in[:], tt_in[:], 0.0, None, op0=Alu.is_gt
                )
                ttp = psB.tile([128, CHUNK], f32, tag="tt")
                for s in range(SUBS):
                    # temp^T subtile (token x tag), pre-relu
                    nc.tensor.matmul(
                        ttp[:, ts(s, 128)], tt_in[:, ts(s, 128)], cmat[:]
                    )
                tempT = workp.tile([128, CHUNK], f32, tag="tempT")
                nc.vector.tensor_scalar_max(tempT[:], ttp[:], 0.0)  # relu

                for s in range(SUBS):
                    i = j * SUBS + s
                    first = i == 0
                    last = i == NSUB - 1
                    xt = xp.tile([128, D], f32, tag="xt")
                    nc.sync.dma_start(xt[:], x_d[ts(i, 128), :])
                    lhs = tempT[:, ts(s, 128)]
                    nc.tensor.matmul(
                        p0[:], lhs, xt[:, 0:512], start=first, stop=last
                    )
                    nc.tensor.matmul(
                        p1[:], lhs, xt[:, 512:1024], start=first, stop=last
                    )
                    nc.tensor.matmul(
                        r_ps[:], onescol[:], lhs, start=first, stop=last
                    )

            # ---- combine partials across cores ----
            P_loc = workp.tile([128, D], f32, tag="Ploc")
            nc.vector.tensor_copy(P_loc[:, 0:512], p0[:])
            nc.vector.tensor_copy(P_loc[:, 512:1024], p1[:])
            r_row = workp.tile([1, 128], f32, tag="rrow")
            nc.vector.tensor_copy(r_row[:], r_ps[:])

            cc_in = dramp.tile([CC_LEN], f32, tag="ccin")
            cc_out = dramp.tile([CC_LEN], f32, tag="ccout")
            cc_in_P = cc_in[0 : T * D].rearrange("(p n) -> p n", p=T)
            cc_in_r = cc_in[T * D : CC_LEN].rearrange("(o t) -> o t", o=1)
            nc.sync.dma_start(cc_in_P, P_loc[:])
            nc.sync.dma_start(cc_in_r, r_row[:])
            if with_cc:
                nc.gpsimd.collective_compute(
                    "AllReduce",
                    Alu.add,
                    replica_groups=[list(range(NCORES))],
                    ins=[cc_in[:].opt()],
                    outs=[cc_out[:].opt()],
                )
            else:
                # TimelineSim-only variant: keep the DRAM round-trip, skip CC
                nc.sync.dma_start(cc_out[:], cc_in[:])

            P_sb = workp.tile([128, D], f32, tag="Pglob")
            nc.sync.dma_start(P_sb[:], cc_out[0 : T * D].rearrange("(p n) -> p n", p=T))
            r_col = workp.tile([128, 1], f32, tag="rcol")
            nc.sync.dma_start(
                r_col[:], cc_out[T * D : CC_LEN].rearrange("(t o) -> t o", o=1)
            )
            inv_r = workp.tile([128, 1], f32, tag="invr")
            nc.vector.reciprocal(inv_r[:], r_col[:])
            nc.vector.tensor_scalar_mul(P_sb[:], P_sb[:], inv_r[:])

            # ---- out = M @ (diag(1/r) P)  (lhsT = MT) ----
            out_sb = workp.tile([128, D], f32, tag="outsb")
            for h in range(2):
                o_ps = psB.tile([128, 512], f32, tag="tt")
                nc.tensor.matmul(o_ps[:], MT[:], P_sb[:, ts(h, 512)])
                nc.vector.tensor_copy(out_sb[:, ts(h, 512)], o_ps[:])
            nc.sync.dma_start(out_d[:], out_sb[:])

    nc.compile()
    return nc


def _get_program(with_cc=True):
    if with_cc not in _PROGRAM:
        _PROGRAM[with_cc] = _build_program(with_cc)
    return _PROGRAM[with_cc]


def _make_in_maps(inputs):
    x = np.asarray(inputs["inputs"], dtype=np.float32).reshape(N, D)
    t2t = np.asarray(inputs["tag_to_token"], dtype=np.float32).reshape(T, N)
    gm = np.asarray(inputs["gat_mask"], dtype=np.int32).reshape(T, T)
    consts = _host_consts()
    in_maps = []
    for c in range(NCORES):
        lo = c * NS
        m = {
            "x": np.ascontiguousarray(x[lo : lo + NS, :]),
            "t2t": np.ascontiguousarray(t2t[:, lo : lo + NS]),
            "gm": gm,
        }
        m.update(consts)
        in_maps.append(m)
    return in_maps


def _run(inputs, trace=False, **kw):
    from concourse.bass_utils import run_bass_kernel_spmd

    nc = _get_program()
    in_maps = _make_in_maps(inputs)
    res = run_bass_kernel_spmd(
        nc, in_maps, list(range(NCORES)), trace=trace, **kw
    )
    out = np.asarray(res.results[0]["out"]).reshape(B, T, D).astype(np.float32)
    return out, res


def kernel(**inputs) -> np.ndarray:
    out, _ = _run(inputs, trace=False)
    return out
